# revision 6
# baseline (speedup 1.0000x reference)
"""Batched Bjorck orthogonalization (512 x 256 x 256, 7 iters) on 8 TRN2 cores.

Per-matrix recurrence (beta=0.5):
    A = W^T W
    W <- W @ (1.5 I - 0.5 A)

Implementation notes:
  - Batch dim (512) sharded across 8 cores -> 64 matrices/core, no comms.
  - Dual state (W, V=W^T) avoids transposes: with M = 1.5I - 0.5A,
        A  = W^T W      (lhsT=W chunk,  rhs=W)
        W' = V^T M      (lhsT=V chunk,  rhs=M)   [= W M]
        V' = M^T V      (lhsT=M chunk,  rhs=V)   [= M V = W'^T, M symmetric]
    All matmuls are [128x128] @ [128x256] in float32r (full PE rate at N=256).
  - M is built in ONE DVE op: scalar_tensor_tensor(out, A_psum, -0.5, 1.5I).
  - Each 256x256 operand lives in SBUF as [128, 2, 256] (row chunks).
  - Each matmul product accumulates in one PSUM bank ([128, 2, 256] tile),
    single accumulation group per bank (start on first mm, stop on last).
"""

import numpy as np

N_CORES = 8
B_FULL = 512
N = 256
NITERS = 7
BETA = 0.5

_CACHE = {}


def _build_nc(n_mats, n_iters=NITERS):
    import concourse.bass as bass  # noqa: F401
    import concourse.mybir as mybir
    from concourse import bacc
    from concourse.tile import TileContext
    from concourse.masks import make_identity
    from concourse.bass import ds

    F32 = mybir.dt.float32
    F32R = mybir.dt.float32r
    ADD = mybir.AluOpType.add
    MULT = mybir.AluOpType.mult

    nc = bacc.Bacc(None, target_bir_lowering=False)
    w_in = nc.declare_dram_parameter("w", [n_mats, N, N], F32R, isOutput=False)
    w_out = nc.declare_dram_parameter("out", [n_mats, N, N], F32R, isOutput=True)

    def mm_group(psum, lhs_tile, rhs_tile):
        # psum[:, m, :] = sum_k lhs_tile[:, k, 128m:128m+128]^T @ rhs_tile[:, k, :]
        n_mm = 0
        for m in range(2):
            for k in range(2):
                nc.tensor.matmul(
                    psum[:, m, :],
                    lhsT=lhs_tile[:, k, ds(128 * m, 128)],
                    rhs=rhs_tile[:, k, :],
                    start=(n_mm == 0),
                    stop=(n_mm == 3),
                )
                n_mm += 1

    with TileContext(nc) as tc:
        with (
            tc.tile_pool(name="const", bufs=1) as cpool,
            tc.tile_pool(name="state", bufs=3) as spool,
            tc.tile_pool(name="psum", bufs=2, space="PSUM") as ppool,
        ):
            id128 = cpool.tile([128, 128], F32, name="id128")
            make_identity(nc, id128)
            idstage = cpool.tile([128, 2, N], F32, name="idstage")
            nc.vector.memset(idstage[:], 0.0)
            nc.vector.tensor_copy(idstage[:, 0, 0:128], id128[:])
            nc.vector.tensor_copy(idstage[:, 1, 128:256], id128[:])
            ident = cpool.tile([128, 2, N], F32R, name="ident")
            nc.vector.tensor_copy(ident[:], idstage[:])
            id15 = cpool.tile([128, 2, N], F32R, name="id15")
            nc.vector.tensor_scalar_mul(id15[:], idstage[:], 1.0 + BETA)

            GROUP = 2  # matrices emitted interleaved, for cross-matrix overlap
            for g0 in range(0, n_mats, GROUP):
                mats = range(g0, min(g0 + GROUP, n_mats))
                W = {}
                V = {}
                for mat in mats:
                    Wsb = W[mat] = spool.tile(
                        [128, 2, N], F32R, name=f"W_{mat}", tag="W", bufs=4
                    )
                    nc.sync.dma_start(
                        Wsb[:], w_in[mat].rearrange("(c p) n -> p c n", p=128)
                    )
                for mat in mats:
                    # V0 = W^T  (matmul against identity)
                    psumV = ppool.tile(
                        [128, 2, N], F32, name=f"pV_{mat}", tag="pV", bufs=3
                    )
                    mm_group(psumV, W[mat], ident)
                    V[mat] = spool.tile(
                        [128, 2, N], F32R, name=f"V_{mat}", tag="V", bufs=4
                    )
                    nc.scalar.copy(V[mat][:], psumV[:])

                for t in range(n_iters):
                    last = t == n_iters - 1
                    for mat in mats:
                        psumA = ppool.tile(
                            [128, 2, N], F32, name=f"pA_{mat}_{t}", tag="pA", bufs=2
                        )
                        mm_group(psumA, W[mat], W[mat])
                        Msb = spool.tile(
                            [128, 2, N], F32R, name=f"M_{mat}_{t}", tag="M", bufs=4
                        )
                        nc.vector.scalar_tensor_tensor(
                            out=Msb[:],
                            in0=psumA[:],
                            scalar=-BETA,
                            in1=id15[:],
                            op0=MULT,
                            op1=ADD,
                        )
                        psumW = ppool.tile(
                            [128, 2, N], F32, name=f"pW_{mat}_{t}", tag="pW", bufs=3
                        )
                        mm_group(psumW, V[mat], Msb)
                        newW = spool.tile(
                            [128, 2, N], F32R, name=f"Wn_{mat}_{t}", tag="W", bufs=4
                        )
                        nc.vector.tensor_copy(newW[:], psumW[:])
                        if not last:
                            psumV2 = ppool.tile(
                                [128, 2, N],
                                F32,
                                name=f"pV2_{mat}_{t}",
                                tag="pV",
                                bufs=3,
                            )
                            mm_group(psumV2, Msb, V[mat])
                            newV = spool.tile(
                                [128, 2, N],
                                F32R,
                                name=f"Vn_{mat}_{t}",
                                tag="V",
                                bufs=4,
                            )
                            nc.scalar.copy(newV[:], psumV2[:])
                            V[mat] = newV
                        W[mat] = newW

                for mat in mats:
                    nc.sync.dma_start(
                        w_out[mat].rearrange("(c p) n -> p c n", p=128), W[mat][:]
                    )
    nc.finalize()
    return nc


def _run_spmd(w, trace=False):
    from concourse.bass_utils import run_bass_kernel_spmd

    w = np.ascontiguousarray(w, dtype=np.float32)
    b = w.shape[0]
    n_mats = b // N_CORES
    key = (n_mats,)
    if key not in _CACHE:
        _CACHE[key] = _build_nc(n_mats)
    nc = _CACHE[key]

    shards = w.reshape(N_CORES, n_mats, N, N)
    in_maps = [{"w": shards[i]} for i in range(N_CORES)]
    res = run_bass_kernel_spmd(
        nc, in_maps, core_ids=list(range(N_CORES)), trace=trace
    )
    out = np.concatenate([res.results[i]["out"] for i in range(N_CORES)], axis=0)
    return out.reshape(b, N, N).astype(np.float32), res


def kernel(w):
    out, _ = _run_spmd(w, trace=False)
    return out
